# revision 37
# baseline (speedup 1.0000x reference)
"""HB-LSTM cell fused Trainium2 kernel, data-parallel over 8 NeuronCores.

Computes, for gate order (f, i, o, u, k):
    pre  = x @ Wx[g].T + bx[g] + h_prev @ Uh[g].T + bh[g]
    f,i,o,u = sigmoid(pre[0..3]);  c = tanh(pre[4])
    kp = u*c + (1-u)*kp_prev
    k  = f*k_prev + i*kp
    h  = o*tanh(k)
Returns (h, k, kp), each [B, H] float32.

Sharding: batch dim B=65536 split across 8 cores (8192 rows each); weight
stacks replicated to every core.

Per-core structure (64 b-tiles of 128 rows, groups of GROUP=8):
  - fp16 on-chip compute throughout: same PE/DVE throughput as bf16 but 8x
    the mantissa, so the fused tail stays well inside the error budget.
  - All casting DMAs (x/h/k/kp loads fp32->fp16, output stores fp16->fp32)
    on the Pool SWDGE ring; stores for group g issue after loads of g+1 so
    Pool never stalls the load pipeline.
  - x/h transposed to feature-major on the PE (identity matmul, fp16) into
    1-bank PSUM tiles; DVE 2x-mode copies them to SBUF lhsT tiles.
  - 5-gate pre-activations accumulate in one [128,1280] PSUM tile per
    b-tile; bias pre-fills the tile (K=1 ones-matmul on PE for cols 0:512,
    DVE broadcast copy for 512:1280), then 12 fp16 matmuls accumulate both
    GEMMs on top.
  - ACT: one wide sigmoid [128,1024] + tanh [128,256] per b-tile, fp16 out.
  - Elementwise tail in fp16 on DVE (2x mode), batched per half-group.
"""

import contextlib

import numpy as np

import concourse.bacc as bacc
import concourse.mybir as mybir
from concourse import tile
from concourse.bass_utils import run_bass_kernel_spmd

N_CORES = 8
B = 65536
IN = 256
H = 256
G5 = 5
BL = B // N_CORES          # rows per core
NT = BL // 128             # 64 b-tiles per core
GROUP = 8                  # b-tiles per DMA group
NG = NT // GROUP
DG = G5 * H                # 1280 = all-gate column span
F32 = mybir.dt.float32
F16 = mybir.dt.float16
F8 = mybir.dt.float8e4
AF = mybir.ActivationFunctionType
DR = mybir.MatmulPerfMode.DoubleRow

# Bench mode: when set, the main loop runs LOOP_N times inside a hardware
# For_i loop so device time dominates RPC overhead in wall-clock.
LOOP_N = None

# Probe mode for HW decomposition benches: None = full kernel,
# "pe" = loads + transposes + matmuls only, "dma" = loads + stores only,
# "dma32" = fp32 HWDGE loads + stores only (no casts, no Pool).
PROBE = None

# Experiment knobs for probing HW behavior.
GEMM_DT = None          # overrides F16 for the GEMM datapath if set
BIAS_MODE = "dr"        # "dr" = fp8 DoubleRow, "ones" = fp16 K=1 matmul
TMODE = "pe"            # activation transposes: "pe" matmul or "dma" xbar

_CACHE = {}


def _build():
    if "nc" in _CACHE:
        return _CACHE["nc"]
    GD = GEMM_DT or F16

    nc = bacc.Bacc("TRN2", target_bir_lowering=False, debug=False,
                   num_devices=N_CORES)

    x_d = nc.dram_tensor("x", [BL, IN], F32, kind="ExternalInput")
    h_d = nc.dram_tensor("h_prev", [BL, H], F32, kind="ExternalInput")
    k_d = nc.dram_tensor("k_prev", [BL, H], F32, kind="ExternalInput")
    kp_d = nc.dram_tensor("kp_prev", [BL, H], F32, kind="ExternalInput")
    wx_d = nc.dram_tensor("Wx", [G5, H, IN], F32, kind="ExternalInput")
    bx_d = nc.dram_tensor("bx", [G5, H], F32, kind="ExternalInput")
    uh_d = nc.dram_tensor("Uh", [G5, H, H], F32, kind="ExternalInput")
    bh_d = nc.dram_tensor("bh", [G5, H], F32, kind="ExternalInput")
    ho_d = nc.dram_tensor("h_out", [BL, H], F32, kind="ExternalOutput")
    ko_d = nc.dram_tensor("k_out", [BL, H], F32, kind="ExternalOutput")
    kpo_d = nc.dram_tensor("kp_out", [BL, H], F32, kind="ExternalOutput")

    with tile.TileContext(nc) as tc:
        with tc.tile_pool(name="const", bufs=1) as cpool:
            # WT[(side, c)]: [128 (i-chunk c), 1280 (g,h)] bf16 = matmul rhs
            WT = {}
            for side in ("x", "h"):
                for c in range(2):
                    WT[side, c] = cpool.tile([128, DG], GD,
                                             name=f"WT_{side}{c}",
                                             tag=f"WT_{side}{c}")
            # fp8 DoubleRow bias operands: bias = 0.5*r0 + 0.5*r1 where
            # r0 = fp8(2b), r1 = fp8(2b - r0) (residual encoding, err ~2e-4)
            ones8 = cpool.tile([1, 2, 128], F8, tag="ones8")
            b8 = cpool.tile([1, 2, DG], F8, tag="b8")
            ident = cpool.tile([128, 128], GD, tag="ident")

            with tc.tile_pool(name="binit", bufs=1) as bpool:
                # identity matrix for PE transposes, built first so the Pool
                # engine frees up for the group-0 activation loads
                onesq = bpool.tile([128, 128], GD, tag="onesq")
                nc.vector.memset(onesq[:], 1.0)
                nc.gpsimd.affine_select(
                    ident[:], onesq[:], pattern=[[-1, 128]], base=0,
                    channel_multiplier=1,
                    compare_op=mybir.AluOpType.is_equal, fill=0.0)
                nc.vector.memset(ones8[:], 0.5)

                # fp32 identity for the weight PE transposes
                ident32 = bpool.tile([128, 128], F32, tag="ident32")
                onesq32 = bpool.tile([128, 128], F32, tag="onesq32")
                nc.vector.memset(onesq32[:], 1.0)
                nc.gpsimd.affine_select(
                    ident32[:], onesq32[:], pattern=[[-1, 128]], base=0,
                    channel_multiplier=1,
                    compare_op=mybir.AluOpType.is_equal, fill=0.0)

                # --- weights: fp32 HWDGE loads (SP/ACT in parallel), PE
                # transposes to feature-major, DVE copies cast to fp16 ---
                with tc.tile_pool(name="wload", bufs=1) as wload, \
                     tc.tile_pool(name="wps", bufs=2, space="PSUM") as wps:
                    w32 = {}
                    for side, w_d, eng in (("x", wx_d, nc.sync),
                                           ("h", uh_d, nc.scalar)):
                        w32[side] = wload.tile([128, 2 * G5, IN], F32,
                                               name=f"w32{side}",
                                               tag=f"w32{side}")
                        wsrc = w_d.ap().rearrange(
                            "g (hc p) i -> p (g hc) i", p=128)
                        eng.dma_start(w32[side][:], wsrc)
                    for side in ("x", "h"):
                        for c in range(2):
                            for gh0 in (0, 5):
                                pt32 = wps.tile([128, 5, 128], F32,
                                                tag="pt32")
                                for t in range(5):
                                    nc.tensor.transpose(
                                        pt32[:, t, :],
                                        w32[side][:, gh0 + t,
                                                  c * 128:(c + 1) * 128],
                                        ident32[:])
                                nc.scalar.copy(
                                    WT[side, c][:, gh0 * 128:
                                                (gh0 + 5) * 128],
                                    pt32[:])

                ones16 = cpool.tile([1, 128], F16, tag="ones16")
                nc.vector.memset(ones16[:], 1.0)
                bs16 = cpool.tile([1, DG], F16, tag="bs16")
                # --- bias rows (fp8 residual pair for DoubleRow matmul) ---
                bxr = bpool.tile([G5, H], F32, tag="bxr")
                nc.scalar.dma_start(bxr[:], bx_d.ap())
                bhr = bpool.tile([G5, H], F32, tag="bhr")
                nc.scalar.dma_start(bhr[:], bh_d.ap())
                bsr = bpool.tile([G5, H], F32, tag="bsr")
                nc.vector.tensor_add(bsr[:], bxr[:], bhr[:])
                bsd = bpool.tile([G5, H], F32, tag="bsd")
                nc.vector.tensor_scalar_mul(bsd[:], bsr[:], 2.0)
                r0 = bpool.tile([G5, H], F8, tag="r0")
                nc.vector.tensor_copy(r0[:], bsd[:])
                res = bpool.tile([G5, H], F32, tag="res")
                nc.vector.tensor_sub(res[:], bsd[:], r0[:])
                r1 = bpool.tile([G5, H], F8, tag="r1")
                nc.vector.tensor_copy(r1[:], res[:])
                # flatten [5,256] -> one row [1,1280] (partition-major)
                nc.scalar.dma_start(b8[:, 0, :], r0[:])
                nc.scalar.dma_start(b8[:, 1, :], r1[:])
                bsg = bpool.tile([G5, H], F16, tag="bsg")
                nc.vector.tensor_copy(bsg[:], bsr[:])
                nc.scalar.dma_start(bs16[:], bsg[:])

            # --- main loop ---
            x_t = x_d.ap().rearrange("(n p) i -> p n i", p=128)
            h_t = h_d.ap().rearrange("(n p) i -> p n i", p=128)
            k_t = k_d.ap().rearrange("(n p) i -> p n i", p=128)
            kp_t = kp_d.ap().rearrange("(n p) i -> p n i", p=128)
            ho_t = ho_d.ap().rearrange("(n p) i -> p n i", p=128)
            ko_t = ko_d.ap().rearrange("(n p) i -> p n i", p=128)
            kpo_t = kpo_d.ap().rearrange("(n p) i -> p n i", p=128)

            loop_cm = (tc.For_i(0, LOOP_N, 1) if LOOP_N
                       else contextlib.nullcontext())
            with tc.tile_pool(name="io", bufs=2) as io, \
                 tc.tile_pool(name="work", bufs=2) as work, \
                 tc.tile_pool(name="tmp", bufs=1) as tmp, \
                 tc.tile_pool(name="psum", bufs=2, space="PSUM") as pp, \
                 tc.tile_pool(name="psumt", bufs=2, space="PSUM") as ppx, \
                 loop_cm:
                x_cm = x_d.ap().rearrange("(n p) (c q) -> p c n q",
                                          p=128, q=128)
                h_cm = h_d.ap().rearrange("(n p) (c q) -> p c n q",
                                          p=128, q=128)
                prev = None
                if PROBE in ("dma8k", "dma4kc"):
                    f = 8 if PROBE == "dma8k" else 4
                    fw = f * 256
                    n_per = GROUP // f
                    dt_l = F32 if PROBE == "dma8k" else F16
                    aps = {}
                    for nm, t_d in (("x", x_d), ("h", h_d), ("k", k_d),
                                    ("kp", kp_d), ("ho", ho_d), ("ko", ko_d),
                                    ("kpo", kpo_d)):
                        aps[nm] = t_d.ap().rearrange(
                            "(n p f) i -> p n (f i)", p=128, f=f)
                    eng_l = nc.gpsimd if PROBE == "dma4kc" else nc.sync
                    eng_l2 = nc.gpsimd if PROBE == "dma4kc" else nc.scalar
                    for gi in range(NG):
                        nsl = slice(gi * n_per, (gi + 1) * n_per)
                        x32p = io.tile([128, n_per, fw], dt_l, tag="x32q")
                        eng_l.dma_start(x32p[:], aps["x"][:, nsl, :])
                        h32p = io.tile([128, n_per, fw], dt_l, tag="h32q")
                        eng_l.dma_start(h32p[:], aps["h"][:, nsl, :])
                        k32p = io.tile([128, n_per, fw], dt_l, tag="k32q")
                        eng_l2.dma_start(k32p[:], aps["k"][:, nsl, :])
                        kp32p = io.tile([128, n_per, fw], dt_l, tag="kp32q")
                        eng_l2.dma_start(kp32p[:], aps["kp"][:, nsl, :])
                        if prev is not None:
                            pk, pkp, px, psl = prev
                            eng_l.dma_start(aps["kpo"][:, psl, :], pk[:])
                            eng_l2.dma_start(aps["ko"][:, psl, :], pkp[:])
                            eng_l.dma_start(aps["ho"][:, psl, :], px[:])
                        prev = (k32p, kp32p, x32p, nsl)
                if PROBE == "dma4k":
                    # 4 consecutive rows per partition -> 4KB HBM descriptors
                    x_t4 = x_d.ap().rearrange("(n p f) i -> p n (f i)",
                                              p=128, f=4)
                    h_t4 = h_d.ap().rearrange("(n p f) i -> p n (f i)",
                                              p=128, f=4)
                    k_t4 = k_d.ap().rearrange("(n p f) i -> p n (f i)",
                                              p=128, f=4)
                    kp_t4 = kp_d.ap().rearrange("(n p f) i -> p n (f i)",
                                                p=128, f=4)
                    ho_t4 = ho_d.ap().rearrange("(n p f) i -> p n (f i)",
                                                p=128, f=4)
                    ko_t4 = ko_d.ap().rearrange("(n p f) i -> p n (f i)",
                                                p=128, f=4)
                    kpo_t4 = kpo_d.ap().rearrange("(n p f) i -> p n (f i)",
                                                  p=128, f=4)
                    for gi in range(NG):
                        nsl = slice(gi * 2, gi * 2 + 2)
                        x32p = io.tile([128, 2, 1024], F32, tag="x32p")
                        nc.sync.dma_start(x32p[:], x_t4[:, nsl, :])
                        h32p = io.tile([128, 2, 1024], F32, tag="h32p")
                        nc.sync.dma_start(h32p[:], h_t4[:, nsl, :])
                        k32p = io.tile([128, 2, 1024], F32, tag="k32p")
                        nc.scalar.dma_start(k32p[:], k_t4[:, nsl, :])
                        kp32p = io.tile([128, 2, 1024], F32, tag="kp32p")
                        nc.scalar.dma_start(kp32p[:], kp_t4[:, nsl, :])
                        if prev is not None:
                            pk, pkp, px, psl = prev
                            nc.sync.dma_start(kpo_t4[:, psl, :], pk[:])
                            nc.scalar.dma_start(ko_t4[:, psl, :], pkp[:])
                            nc.sync.dma_start(ho_t4[:, psl, :], px[:])
                        prev = (k32p, kp32p, x32p, nsl)
                if PROBE == "dma32":
                    for gi in range(NG):
                        nsl = slice(gi * GROUP, (gi + 1) * GROUP)
                        x32p = io.tile([128, GROUP, IN], F32, tag="x32p")
                        nc.sync.dma_start(x32p[:], x_t[:, nsl, :])
                        h32p = io.tile([128, GROUP, H], F32, tag="h32p")
                        nc.sync.dma_start(h32p[:], h_t[:, nsl, :])
                        k32p = io.tile([128, GROUP, H], F32, tag="k32p")
                        nc.scalar.dma_start(k32p[:], k_t[:, nsl, :])
                        kp32p = io.tile([128, GROUP, H], F32, tag="kp32p")
                        nc.scalar.dma_start(kp32p[:], kp_t[:, nsl, :])
                        if prev is not None:
                            pk, pkp, px, psl = prev
                            nc.sync.dma_start(kpo_t[:, psl, :], pk[:])
                            nc.scalar.dma_start(ko_t[:, psl, :], pkp[:])
                            nc.sync.dma_start(ho_t[:, psl, :], px[:])
                        prev = (k32p, kp32p, x32p, nsl)
                for gi in range(0 if PROBE in ("dma32", "dma4k", "dma8k",
                                               "dma4kc") else NG):
                    nsl = slice(gi * GROUP, (gi + 1) * GROUP)
                    # Pool SWDGE cast loads (fp32 -> fp16 in flight)
                    if PROBE != "mm":
                        if TMODE == "dma":
                            x16 = io.tile([128, 2, GROUP, 128], GD,
                                          tag="x16")
                            nc.gpsimd.dma_start(x16[:], x_cm[:, :, nsl, :])
                            h16 = io.tile([128, 2, GROUP, 128], GD,
                                          tag="h16")
                            nc.gpsimd.dma_start(h16[:], h_cm[:, :, nsl, :])
                        else:
                            x16 = io.tile([128, GROUP, IN], GD, tag="x16")
                            nc.gpsimd.dma_start(x16[:], x_t[:, nsl, :])
                            h16 = io.tile([128, GROUP, H], GD, tag="h16")
                            nc.gpsimd.dma_start(h16[:], h_t[:, nsl, :])
                    if PROBE not in ("pe", "mm", "mmload"):
                        kpr16 = io.tile([128, GROUP, H], F16, tag="kpr16")
                        nc.gpsimd.dma_start(kpr16[:], k_t[:, nsl, :])
                        kppr16 = io.tile([128, GROUP, H], F16, tag="kppr16")
                        nc.gpsimd.dma_start(kppr16[:], kp_t[:, nsl, :])
                    # previous group's stores (fp16->fp32 cast on Pool SWDGE)
                    if prev is not None:
                        pkp, pk, ph, psl = prev
                        nc.gpsimd.dma_start(kpo_t[:, psl, :], pkp[:])
                        nc.gpsimd.dma_start(ko_t[:, psl, :], pk[:])
                        nc.gpsimd.dma_start(ho_t[:, psl, :], ph[:])
                    if PROBE == "dma":
                        prev = (kpr16, kppr16, x16, nsl)
                        continue

                    # PE transposes -> 1-bank PSUM tiles -> DVE 2x copies
                    if PROBE in ("mm", "mmload"):
                        if "xTs" not in _CACHE:
                            _CACHE["xTs"] = cpool.tile(
                                [128, 2, GROUP, 128], GD, tag="xTs",
                                name="xTs")
                            _CACHE["hTs"] = cpool.tile(
                                [128, 2, GROUP, 128], GD, tag="hTs",
                                name="hTs")
                            nc.vector.memset(_CACHE["xTs"][:], 0.0)
                            nc.vector.memset(_CACHE["hTs"][:], 0.0)
                        xT = _CACHE["xTs"]
                        hT = _CACHE["hTs"]
                    elif TMODE == "dma":
                        xT = work.tile([128, 2, GROUP, 128], GD, tag="xT")
                        hT = work.tile([128, 2, GROUP, 128], GD, tag="hT")
                        for c in range(2):
                            nc.sync.dma_start(xT[:, c], x16[:, c],
                                              transpose=True)
                            nc.scalar.dma_start(hT[:, c], h16[:, c],
                                                transpose=True)
                    else:
                        xT = work.tile([128, 2, GROUP, 128], GD, tag="xT")
                        hT = work.tile([128, 2, GROUP, 128], GD, tag="hT")
                        for a16, aT in ((x16, xT), (h16, hT)):
                            for c in range(2):
                                pt = ppx.tile([128, GROUP, 128], GD,
                                              tag="pt")
                                for j in range(GROUP):
                                    nc.tensor.transpose(
                                        pt[:, j, :],
                                        a16[:, j, c * 128:(c + 1) * 128],
                                        ident[:])
                                nc.vector.tensor_copy(aT[:, c], pt[:])

                    gates = work.tile([128, GROUP, 1024], F16, tag="gates")
                    cg = work.tile([128, GROUP, 256], F16, tag="cg")
                    kp_o = io.tile([128, GROUP, H], F16, tag="kp_o")
                    k_o = io.tile([128, GROUP, H], F16, tag="k_o")
                    h_o = io.tile([128, GROUP, H], F16, tag="h_o")
                    HG = GROUP // 2
                    for half in range(2):
                        for j in range(half * HG, (half + 1) * HG):
                            ps = pp.tile([128, DG], F32, tag="ps")
                            # bias pre-fill: fp8 DoubleRow ones-matmuls
                            # (0.5 cyc/col), one per PSUM bank
                            for n0 in range(0, DG, 512):
                                n1 = min(n0 + 512, DG)
                                if BIAS_MODE == "dr":
                                    nc.tensor.matmul(ps[:, n0:n1], ones8[:],
                                                     b8[:, :, n0:n1],
                                                     start=True, stop=False,
                                                     perf_mode=DR)
                                else:
                                    nc.tensor.matmul(ps[:, n0:n1], ones16[:],
                                                     bs16[:, n0:n1],
                                                     start=True, stop=False)
                            for si, (side, aT) in enumerate((("x", xT),
                                                             ("h", hT))):
                                for c in range(2):
                                    lhsT = aT[:, c, j, :]
                                    last = si == 1 and c == 1
                                    for n0 in range(0, DG, 512):
                                        n1 = min(n0 + 512, DG)
                                        nc.tensor.matmul(
                                            ps[:, n0:n1], lhsT,
                                            WT[side, c][:, n0:n1],
                                            start=False, stop=last,
                                            skip_group_check=True)
                            if PROBE in ("pe", "mm", "mmload"):
                                continue
                            nc.scalar.activation(gates[:, j, :],
                                                 ps[:, 0:1024], AF.Sigmoid)
                            nc.scalar.activation(cg[:, j, :],
                                                 ps[:, 1024:DG], AF.Tanh)

                        if PROBE in ("pe", "mm", "mmload"):
                            continue
                        # fp16 elementwise tail for this half-group (DVE 2x)
                        hsl = slice(half * HG, (half + 1) * HG)
                        f_ = gates[:, hsl, 0:256]
                        i_ = gates[:, hsl, 256:512]
                        o_ = gates[:, hsl, 512:768]
                        u_ = gates[:, hsl, 768:1024]
                        kpp_h = kppr16[:, hsl, :]
                        d = tmp.tile([128, HG, H], F16, tag="d")
                        nc.vector.tensor_sub(d[:], cg[:, hsl, :], kpp_h)
                        e = tmp.tile([128, HG, H], F16, tag="e")
                        nc.vector.tensor_mul(e[:], u_, d[:])
                        m = tmp.tile([128, HG, H], F16, tag="m")
                        nc.vector.tensor_mul(m[:], f_, kpr16[:, hsl, :])
                        nc.vector.tensor_add(kp_o[:, hsl, :], e[:], kpp_h)
                        n2 = tmp.tile([128, HG, H], F16, tag="n2")
                        nc.vector.tensor_mul(n2[:], i_, kp_o[:, hsl, :])
                        nc.vector.tensor_add(k_o[:, hsl, :], m[:], n2[:])
                        tk = tmp.tile([128, HG, H], F16, tag="tk")
                        nc.scalar.activation(tk[:], k_o[:, hsl, :], AF.Tanh)
                        nc.vector.tensor_mul(h_o[:, hsl, :], o_, tk[:])

                        if gi == NG - 1:
                            # last group: store each half as soon as ready
                            gsl = slice(gi * GROUP + half * HG,
                                        gi * GROUP + (half + 1) * HG)
                            nc.gpsimd.dma_start(kpo_t[:, gsl, :],
                                                kp_o[:, hsl, :])
                            nc.gpsimd.dma_start(ko_t[:, gsl, :],
                                                k_o[:, hsl, :])
                            nc.gpsimd.dma_start(ho_t[:, gsl, :],
                                                h_o[:, hsl, :])

                    if PROBE in ("pe", "mm", "mmload"):
                        prev = None
                    else:
                        prev = ((kp_o, k_o, h_o, nsl)
                                if gi < NG - 1 else None)

    nc.compile()
    _CACHE["nc"] = nc
    return nc


def kernel(x, h_prev, k_prev, kp_prev, Wx, bx, Uh, bh):
    x = np.asarray(x, dtype=np.float32)
    h_prev = np.asarray(h_prev, dtype=np.float32)
    k_prev = np.asarray(k_prev, dtype=np.float32)
    kp_prev = np.asarray(kp_prev, dtype=np.float32)
    Wx = np.ascontiguousarray(np.asarray(Wx, dtype=np.float32))
    bx = np.ascontiguousarray(np.asarray(bx, dtype=np.float32))
    Uh = np.ascontiguousarray(np.asarray(Uh, dtype=np.float32))
    bh = np.ascontiguousarray(np.asarray(bh, dtype=np.float32))

    nc = _build()
    in_maps = []
    for c in range(N_CORES):
        sl = slice(c * BL, (c + 1) * BL)
        in_maps.append({
            "x": np.ascontiguousarray(x[sl]),
            "h_prev": np.ascontiguousarray(h_prev[sl]),
            "k_prev": np.ascontiguousarray(k_prev[sl]),
            "kp_prev": np.ascontiguousarray(kp_prev[sl]),
            "Wx": Wx, "bx": bx, "Uh": Uh, "bh": bh,
        })
    res = run_bass_kernel_spmd(nc, in_maps, list(range(N_CORES)))
    h_out = np.concatenate([res.results[c]["h_out"] for c in range(N_CORES)],
                           axis=0)
    k_out = np.concatenate([res.results[c]["k_out"] for c in range(N_CORES)],
                           axis=0)
    kp_out = np.concatenate([res.results[c]["kp_out"]
                             for c in range(N_CORES)], axis=0)
    return (h_out, k_out, kp_out)


# revision 38
# speedup vs baseline: 1.1839x; 1.1839x over previous
"""HB-LSTM cell fused Trainium2 kernel, data-parallel over 8 NeuronCores.

Computes, for gate order (f, i, o, u, k):
    pre  = x @ Wx[g].T + bx[g] + h_prev @ Uh[g].T + bh[g]
    f,i,o,u = sigmoid(pre[0..3]);  c = tanh(pre[4])
    kp = u*c + (1-u)*kp_prev
    k  = f*k_prev + i*kp
    h  = o*tanh(k)
Returns (h, k, kp), each [B, H] float32.

Sharding: batch dim B=65536 split across 8 cores (8192 rows each); weight
stacks replicated to every core.

Per-core structure (64 b-tiles of 128 rows, groups of GROUP=8):
  - fp16 on-chip compute throughout: same PE/DVE throughput as bf16 but 8x
    the mantissa, so the fused tail stays well inside the error budget.
  - All casting DMAs (x/h/k/kp loads fp32->fp16, output stores fp16->fp32)
    on the Pool SWDGE ring; stores for group g issue after loads of g+1 so
    Pool never stalls the load pipeline.
  - x/h transposed to feature-major on the PE (identity matmul, fp16) into
    1-bank PSUM tiles; DVE 2x-mode copies them to SBUF lhsT tiles.
  - 5-gate pre-activations accumulate in one [128,1280] PSUM tile per
    b-tile; bias pre-fills the tile (K=1 ones-matmul on PE for cols 0:512,
    DVE broadcast copy for 512:1280), then 12 fp16 matmuls accumulate both
    GEMMs on top.
  - ACT: one wide sigmoid [128,1024] + tanh [128,256] per b-tile, fp16 out.
  - Elementwise tail in fp16 on DVE (2x mode), batched per half-group.
"""

import contextlib

import numpy as np

import concourse.bacc as bacc
import concourse.mybir as mybir
from concourse import tile
from concourse.bass_utils import run_bass_kernel_spmd

N_CORES = 8
B = 65536
IN = 256
H = 256
G5 = 5
BL = B // N_CORES          # rows per core
NT = BL // 128             # 64 b-tiles per core
GROUP = 4                  # b-tiles per DMA group
NG = NT // GROUP
DG = G5 * H                # 1280 = all-gate column span
F32 = mybir.dt.float32
F16 = mybir.dt.float16
F8 = mybir.dt.float8e4
AF = mybir.ActivationFunctionType
DR = mybir.MatmulPerfMode.DoubleRow

# Bench mode: when set, the main loop runs LOOP_N times inside a hardware
# For_i loop so device time dominates RPC overhead in wall-clock.
LOOP_N = None

# Probe mode for HW decomposition benches: None = full kernel,
# "pe" = loads + transposes + matmuls only, "dma" = loads + stores only,
# "dma32" = fp32 HWDGE loads + stores only (no casts, no Pool).
PROBE = None

# Experiment knobs for probing HW behavior.
GEMM_DT = None          # overrides F16 for the GEMM datapath if set
BIAS_MODE = "dr"        # "dr" = fp8 DoubleRow, "ones" = fp16 K=1 matmul
TMODE = "pe"            # activation transposes: "pe" matmul or "dma" xbar

_CACHE = {}


def _build():
    if "nc" in _CACHE:
        return _CACHE["nc"]
    GD = GEMM_DT or F16

    nc = bacc.Bacc("TRN2", target_bir_lowering=False, debug=False,
                   num_devices=N_CORES)

    x_d = nc.dram_tensor("x", [BL, IN], F32, kind="ExternalInput")
    h_d = nc.dram_tensor("h_prev", [BL, H], F32, kind="ExternalInput")
    k_d = nc.dram_tensor("k_prev", [BL, H], F32, kind="ExternalInput")
    kp_d = nc.dram_tensor("kp_prev", [BL, H], F32, kind="ExternalInput")
    wx_d = nc.dram_tensor("Wx", [G5, H, IN], F32, kind="ExternalInput")
    bx_d = nc.dram_tensor("bx", [G5, H], F32, kind="ExternalInput")
    uh_d = nc.dram_tensor("Uh", [G5, H, H], F32, kind="ExternalInput")
    bh_d = nc.dram_tensor("bh", [G5, H], F32, kind="ExternalInput")
    ho_d = nc.dram_tensor("h_out", [BL, H], F32, kind="ExternalOutput")
    ko_d = nc.dram_tensor("k_out", [BL, H], F32, kind="ExternalOutput")
    kpo_d = nc.dram_tensor("kp_out", [BL, H], F32, kind="ExternalOutput")

    with tile.TileContext(nc) as tc:
        with tc.tile_pool(name="const", bufs=1) as cpool:
            # WT[(side, c)]: [128 (i-chunk c), 1280 (g,h)] bf16 = matmul rhs
            WT = {}
            for side in ("x", "h"):
                for c in range(2):
                    WT[side, c] = cpool.tile([128, DG], GD,
                                             name=f"WT_{side}{c}",
                                             tag=f"WT_{side}{c}")
            # fp8 DoubleRow bias operands: bias = 0.5*r0 + 0.5*r1 where
            # r0 = fp8(2b), r1 = fp8(2b - r0) (residual encoding, err ~2e-4)
            ones8 = cpool.tile([1, 2, 128], F8, tag="ones8")
            b8 = cpool.tile([1, 2, DG], F8, tag="b8")
            ident = cpool.tile([128, 128], GD, tag="ident")

            with tc.tile_pool(name="binit", bufs=1) as bpool:
                # identity matrix for PE transposes, built first so the Pool
                # engine frees up for the group-0 activation loads
                onesq = bpool.tile([128, 128], GD, tag="onesq")
                nc.vector.memset(onesq[:], 1.0)
                nc.gpsimd.affine_select(
                    ident[:], onesq[:], pattern=[[-1, 128]], base=0,
                    channel_multiplier=1,
                    compare_op=mybir.AluOpType.is_equal, fill=0.0)
                nc.vector.memset(ones8[:], 0.5)

                # fp32 identity for the weight PE transposes
                ident32 = bpool.tile([128, 128], F32, tag="ident32")
                onesq32 = bpool.tile([128, 128], F32, tag="onesq32")
                nc.vector.memset(onesq32[:], 1.0)
                nc.gpsimd.affine_select(
                    ident32[:], onesq32[:], pattern=[[-1, 128]], base=0,
                    channel_multiplier=1,
                    compare_op=mybir.AluOpType.is_equal, fill=0.0)

                # --- weights: fp32 HWDGE loads (SP/ACT in parallel), PE
                # transposes to feature-major, DVE copies cast to fp16 ---
                with tc.tile_pool(name="wload", bufs=1) as wload, \
                     tc.tile_pool(name="wps", bufs=2, space="PSUM") as wps:
                    w32 = {}
                    for side, w_d, eng in (("x", wx_d, nc.sync),
                                           ("h", uh_d, nc.scalar)):
                        w32[side] = wload.tile([128, 2 * G5, IN], F32,
                                               name=f"w32{side}",
                                               tag=f"w32{side}")
                        wsrc = w_d.ap().rearrange(
                            "g (hc p) i -> p (g hc) i", p=128)
                        eng.dma_start(w32[side][:], wsrc)
                    for side in ("x", "h"):
                        for c in range(2):
                            for gh0 in (0, 5):
                                pt32 = wps.tile([128, 5, 128], F32,
                                                tag="pt32")
                                for t in range(5):
                                    nc.tensor.transpose(
                                        pt32[:, t, :],
                                        w32[side][:, gh0 + t,
                                                  c * 128:(c + 1) * 128],
                                        ident32[:])
                                nc.scalar.copy(
                                    WT[side, c][:, gh0 * 128:
                                                (gh0 + 5) * 128],
                                    pt32[:])

                ones16 = cpool.tile([1, 128], F16, tag="ones16")
                nc.vector.memset(ones16[:], 1.0)
                bs16 = cpool.tile([1, DG], F16, tag="bs16")
                # --- bias rows (fp8 residual pair for DoubleRow matmul) ---
                bxr = bpool.tile([G5, H], F32, tag="bxr")
                nc.scalar.dma_start(bxr[:], bx_d.ap())
                bhr = bpool.tile([G5, H], F32, tag="bhr")
                nc.scalar.dma_start(bhr[:], bh_d.ap())
                bsr = bpool.tile([G5, H], F32, tag="bsr")
                nc.vector.tensor_add(bsr[:], bxr[:], bhr[:])
                bsd = bpool.tile([G5, H], F32, tag="bsd")
                nc.vector.tensor_scalar_mul(bsd[:], bsr[:], 2.0)
                r0 = bpool.tile([G5, H], F8, tag="r0")
                nc.vector.tensor_copy(r0[:], bsd[:])
                res = bpool.tile([G5, H], F32, tag="res")
                nc.vector.tensor_sub(res[:], bsd[:], r0[:])
                r1 = bpool.tile([G5, H], F8, tag="r1")
                nc.vector.tensor_copy(r1[:], res[:])
                # flatten [5,256] -> one row [1,1280] (partition-major)
                nc.scalar.dma_start(b8[:, 0, :], r0[:])
                nc.scalar.dma_start(b8[:, 1, :], r1[:])
                bsg = bpool.tile([G5, H], F16, tag="bsg")
                nc.vector.tensor_copy(bsg[:], bsr[:])
                nc.scalar.dma_start(bs16[:], bsg[:])

            # --- main loop ---
            x_t = x_d.ap().rearrange("(n p) i -> p n i", p=128)
            h_t = h_d.ap().rearrange("(n p) i -> p n i", p=128)
            k_t = k_d.ap().rearrange("(n p) i -> p n i", p=128)
            kp_t = kp_d.ap().rearrange("(n p) i -> p n i", p=128)
            ho_t = ho_d.ap().rearrange("(n p) i -> p n i", p=128)
            ko_t = ko_d.ap().rearrange("(n p) i -> p n i", p=128)
            kpo_t = kpo_d.ap().rearrange("(n p) i -> p n i", p=128)

            loop_cm = (tc.For_i(0, LOOP_N, 1) if LOOP_N
                       else contextlib.nullcontext())
            with tc.tile_pool(name="io", bufs=2) as io, \
                 tc.tile_pool(name="work", bufs=2) as work, \
                 tc.tile_pool(name="tmp", bufs=1) as tmp, \
                 tc.tile_pool(name="psum", bufs=2, space="PSUM") as pp, \
                 tc.tile_pool(name="psumt", bufs=2, space="PSUM") as ppx, \
                 loop_cm:
                x_cm = x_d.ap().rearrange("(n p) (c q) -> p c n q",
                                          p=128, q=128)
                h_cm = h_d.ap().rearrange("(n p) (c q) -> p c n q",
                                          p=128, q=128)
                prev = None
                if PROBE in ("dma8k", "dma4kc"):
                    f = 8 if PROBE == "dma8k" else 4
                    fw = f * 256
                    n_per = GROUP // f
                    dt_l = F32 if PROBE == "dma8k" else F16
                    aps = {}
                    for nm, t_d in (("x", x_d), ("h", h_d), ("k", k_d),
                                    ("kp", kp_d), ("ho", ho_d), ("ko", ko_d),
                                    ("kpo", kpo_d)):
                        aps[nm] = t_d.ap().rearrange(
                            "(n p f) i -> p n (f i)", p=128, f=f)
                    eng_l = nc.gpsimd if PROBE == "dma4kc" else nc.sync
                    eng_l2 = nc.gpsimd if PROBE == "dma4kc" else nc.scalar
                    for gi in range(NG):
                        nsl = slice(gi * n_per, (gi + 1) * n_per)
                        x32p = io.tile([128, n_per, fw], dt_l, tag="x32q")
                        eng_l.dma_start(x32p[:], aps["x"][:, nsl, :])
                        h32p = io.tile([128, n_per, fw], dt_l, tag="h32q")
                        eng_l.dma_start(h32p[:], aps["h"][:, nsl, :])
                        k32p = io.tile([128, n_per, fw], dt_l, tag="k32q")
                        eng_l2.dma_start(k32p[:], aps["k"][:, nsl, :])
                        kp32p = io.tile([128, n_per, fw], dt_l, tag="kp32q")
                        eng_l2.dma_start(kp32p[:], aps["kp"][:, nsl, :])
                        if prev is not None:
                            pk, pkp, px, psl = prev
                            eng_l.dma_start(aps["kpo"][:, psl, :], pk[:])
                            eng_l2.dma_start(aps["ko"][:, psl, :], pkp[:])
                            eng_l.dma_start(aps["ho"][:, psl, :], px[:])
                        prev = (k32p, kp32p, x32p, nsl)
                if PROBE == "dma4k":
                    # 4 consecutive rows per partition -> 4KB HBM descriptors
                    x_t4 = x_d.ap().rearrange("(n p f) i -> p n (f i)",
                                              p=128, f=4)
                    h_t4 = h_d.ap().rearrange("(n p f) i -> p n (f i)",
                                              p=128, f=4)
                    k_t4 = k_d.ap().rearrange("(n p f) i -> p n (f i)",
                                              p=128, f=4)
                    kp_t4 = kp_d.ap().rearrange("(n p f) i -> p n (f i)",
                                                p=128, f=4)
                    ho_t4 = ho_d.ap().rearrange("(n p f) i -> p n (f i)",
                                                p=128, f=4)
                    ko_t4 = ko_d.ap().rearrange("(n p f) i -> p n (f i)",
                                                p=128, f=4)
                    kpo_t4 = kpo_d.ap().rearrange("(n p f) i -> p n (f i)",
                                                  p=128, f=4)
                    for gi in range(NG):
                        nsl = slice(gi * 2, gi * 2 + 2)
                        x32p = io.tile([128, 2, 1024], F32, tag="x32p")
                        nc.sync.dma_start(x32p[:], x_t4[:, nsl, :])
                        h32p = io.tile([128, 2, 1024], F32, tag="h32p")
                        nc.sync.dma_start(h32p[:], h_t4[:, nsl, :])
                        k32p = io.tile([128, 2, 1024], F32, tag="k32p")
                        nc.scalar.dma_start(k32p[:], k_t4[:, nsl, :])
                        kp32p = io.tile([128, 2, 1024], F32, tag="kp32p")
                        nc.scalar.dma_start(kp32p[:], kp_t4[:, nsl, :])
                        if prev is not None:
                            pk, pkp, px, psl = prev
                            nc.sync.dma_start(kpo_t4[:, psl, :], pk[:])
                            nc.scalar.dma_start(ko_t4[:, psl, :], pkp[:])
                            nc.sync.dma_start(ho_t4[:, psl, :], px[:])
                        prev = (k32p, kp32p, x32p, nsl)
                if PROBE == "dma32":
                    for gi in range(NG):
                        nsl = slice(gi * GROUP, (gi + 1) * GROUP)
                        x32p = io.tile([128, GROUP, IN], F32, tag="x32p")
                        nc.sync.dma_start(x32p[:], x_t[:, nsl, :])
                        h32p = io.tile([128, GROUP, H], F32, tag="h32p")
                        nc.sync.dma_start(h32p[:], h_t[:, nsl, :])
                        k32p = io.tile([128, GROUP, H], F32, tag="k32p")
                        nc.scalar.dma_start(k32p[:], k_t[:, nsl, :])
                        kp32p = io.tile([128, GROUP, H], F32, tag="kp32p")
                        nc.scalar.dma_start(kp32p[:], kp_t[:, nsl, :])
                        if prev is not None:
                            pk, pkp, px, psl = prev
                            nc.sync.dma_start(kpo_t[:, psl, :], pk[:])
                            nc.scalar.dma_start(ko_t[:, psl, :], pkp[:])
                            nc.sync.dma_start(ho_t[:, psl, :], px[:])
                        prev = (k32p, kp32p, x32p, nsl)
                for gi in range(0 if PROBE in ("dma32", "dma4k", "dma8k",
                                               "dma4kc") else NG):
                    nsl = slice(gi * GROUP, (gi + 1) * GROUP)
                    # Pool SWDGE cast loads (fp32 -> fp16 in flight)
                    if PROBE != "mm":
                        if TMODE == "dma":
                            x16 = io.tile([128, 2, GROUP, 128], GD,
                                          tag="x16")
                            nc.gpsimd.dma_start(x16[:], x_cm[:, :, nsl, :])
                            h16 = io.tile([128, 2, GROUP, 128], GD,
                                          tag="h16")
                            nc.gpsimd.dma_start(h16[:], h_cm[:, :, nsl, :])
                        else:
                            x16 = io.tile([128, GROUP, IN], GD, tag="x16")
                            nc.gpsimd.dma_start(x16[:], x_t[:, nsl, :])
                            h16 = io.tile([128, GROUP, H], GD, tag="h16")
                            nc.gpsimd.dma_start(h16[:], h_t[:, nsl, :])
                    if PROBE not in ("pe", "mm", "mmload"):
                        kpr16 = io.tile([128, GROUP, H], F16, tag="kpr16")
                        nc.gpsimd.dma_start(kpr16[:], k_t[:, nsl, :])
                        kppr16 = io.tile([128, GROUP, H], F16, tag="kppr16")
                        nc.gpsimd.dma_start(kppr16[:], kp_t[:, nsl, :])
                    # previous group's stores (fp16->fp32 cast on Pool SWDGE)
                    if prev is not None:
                        pkp, pk, ph, psl = prev
                        nc.gpsimd.dma_start(kpo_t[:, psl, :], pkp[:])
                        nc.gpsimd.dma_start(ko_t[:, psl, :], pk[:])
                        nc.gpsimd.dma_start(ho_t[:, psl, :], ph[:])
                    if PROBE == "dma":
                        prev = (kpr16, kppr16, x16, nsl)
                        continue

                    # PE transposes -> 1-bank PSUM tiles -> DVE 2x copies
                    if PROBE in ("mm", "mmload"):
                        if "xTs" not in _CACHE:
                            _CACHE["xTs"] = cpool.tile(
                                [128, 2, GROUP, 128], GD, tag="xTs",
                                name="xTs")
                            _CACHE["hTs"] = cpool.tile(
                                [128, 2, GROUP, 128], GD, tag="hTs",
                                name="hTs")
                            nc.vector.memset(_CACHE["xTs"][:], 0.0)
                            nc.vector.memset(_CACHE["hTs"][:], 0.0)
                        xT = _CACHE["xTs"]
                        hT = _CACHE["hTs"]
                    elif TMODE == "dma":
                        xT = work.tile([128, 2, GROUP, 128], GD, tag="xT")
                        hT = work.tile([128, 2, GROUP, 128], GD, tag="hT")
                        for c in range(2):
                            nc.sync.dma_start(xT[:, c], x16[:, c],
                                              transpose=True)
                            nc.scalar.dma_start(hT[:, c], h16[:, c],
                                                transpose=True)
                    else:
                        xT = work.tile([128, 2, GROUP, 128], GD, tag="xT")
                        hT = work.tile([128, 2, GROUP, 128], GD, tag="hT")
                        for a16, aT in ((x16, xT), (h16, hT)):
                            for c in range(2):
                                pt = ppx.tile([128, GROUP, 128], GD,
                                              tag="pt")
                                for j in range(GROUP):
                                    nc.tensor.transpose(
                                        pt[:, j, :],
                                        a16[:, j, c * 128:(c + 1) * 128],
                                        ident[:])
                                nc.vector.tensor_copy(aT[:, c], pt[:])

                    gates = work.tile([128, GROUP, 1024], F16, tag="gates")
                    cg = work.tile([128, GROUP, 256], F16, tag="cg")
                    kp_o = io.tile([128, GROUP, H], F16, tag="kp_o")
                    k_o = io.tile([128, GROUP, H], F16, tag="k_o")
                    h_o = io.tile([128, GROUP, H], F16, tag="h_o")
                    HG = GROUP // 2
                    for half in range(2):
                        for j in range(half * HG, (half + 1) * HG):
                            ps = pp.tile([128, DG], F32, tag="ps")
                            # bias pre-fill: fp8 DoubleRow ones-matmuls
                            # (0.5 cyc/col), one per PSUM bank
                            for n0 in range(0, DG, 512):
                                n1 = min(n0 + 512, DG)
                                if BIAS_MODE == "dr":
                                    nc.tensor.matmul(ps[:, n0:n1], ones8[:],
                                                     b8[:, :, n0:n1],
                                                     start=True, stop=False,
                                                     perf_mode=DR)
                                else:
                                    nc.tensor.matmul(ps[:, n0:n1], ones16[:],
                                                     bs16[:, n0:n1],
                                                     start=True, stop=False)
                            for si, (side, aT) in enumerate((("x", xT),
                                                             ("h", hT))):
                                for c in range(2):
                                    lhsT = aT[:, c, j, :]
                                    last = si == 1 and c == 1
                                    for n0 in range(0, DG, 512):
                                        n1 = min(n0 + 512, DG)
                                        nc.tensor.matmul(
                                            ps[:, n0:n1], lhsT,
                                            WT[side, c][:, n0:n1],
                                            start=False, stop=last,
                                            skip_group_check=True)
                            if PROBE in ("pe", "mm", "mmload"):
                                continue
                            nc.scalar.activation(gates[:, j, :],
                                                 ps[:, 0:1024], AF.Sigmoid)
                            nc.scalar.activation(cg[:, j, :],
                                                 ps[:, 1024:DG], AF.Tanh)

                        if PROBE in ("pe", "mm", "mmload"):
                            continue
                        # fp16 elementwise tail for this half-group (DVE 2x)
                        hsl = slice(half * HG, (half + 1) * HG)
                        f_ = gates[:, hsl, 0:256]
                        i_ = gates[:, hsl, 256:512]
                        o_ = gates[:, hsl, 512:768]
                        u_ = gates[:, hsl, 768:1024]
                        kpp_h = kppr16[:, hsl, :]
                        d = tmp.tile([128, HG, H], F16, tag="d")
                        nc.vector.tensor_sub(d[:], cg[:, hsl, :], kpp_h)
                        e = tmp.tile([128, HG, H], F16, tag="e")
                        nc.vector.tensor_mul(e[:], u_, d[:])
                        m = tmp.tile([128, HG, H], F16, tag="m")
                        nc.vector.tensor_mul(m[:], f_, kpr16[:, hsl, :])
                        nc.vector.tensor_add(kp_o[:, hsl, :], e[:], kpp_h)
                        n2 = tmp.tile([128, HG, H], F16, tag="n2")
                        nc.vector.tensor_mul(n2[:], i_, kp_o[:, hsl, :])
                        nc.vector.tensor_add(k_o[:, hsl, :], m[:], n2[:])
                        tk = tmp.tile([128, HG, H], F16, tag="tk")
                        nc.scalar.activation(tk[:], k_o[:, hsl, :], AF.Tanh)
                        nc.vector.tensor_mul(h_o[:, hsl, :], o_, tk[:])

                        if gi == NG - 1:
                            # last group: store each half as soon as ready
                            gsl = slice(gi * GROUP + half * HG,
                                        gi * GROUP + (half + 1) * HG)
                            nc.gpsimd.dma_start(kpo_t[:, gsl, :],
                                                kp_o[:, hsl, :])
                            nc.gpsimd.dma_start(ko_t[:, gsl, :],
                                                k_o[:, hsl, :])
                            nc.gpsimd.dma_start(ho_t[:, gsl, :],
                                                h_o[:, hsl, :])

                    if PROBE in ("pe", "mm", "mmload"):
                        prev = None
                    else:
                        prev = ((kp_o, k_o, h_o, nsl)
                                if gi < NG - 1 else None)

    nc.compile()
    _CACHE["nc"] = nc
    return nc


def kernel(x, h_prev, k_prev, kp_prev, Wx, bx, Uh, bh):
    x = np.asarray(x, dtype=np.float32)
    h_prev = np.asarray(h_prev, dtype=np.float32)
    k_prev = np.asarray(k_prev, dtype=np.float32)
    kp_prev = np.asarray(kp_prev, dtype=np.float32)
    Wx = np.ascontiguousarray(np.asarray(Wx, dtype=np.float32))
    bx = np.ascontiguousarray(np.asarray(bx, dtype=np.float32))
    Uh = np.ascontiguousarray(np.asarray(Uh, dtype=np.float32))
    bh = np.ascontiguousarray(np.asarray(bh, dtype=np.float32))

    nc = _build()
    in_maps = []
    for c in range(N_CORES):
        sl = slice(c * BL, (c + 1) * BL)
        in_maps.append({
            "x": np.ascontiguousarray(x[sl]),
            "h_prev": np.ascontiguousarray(h_prev[sl]),
            "k_prev": np.ascontiguousarray(k_prev[sl]),
            "kp_prev": np.ascontiguousarray(kp_prev[sl]),
            "Wx": Wx, "bx": bx, "Uh": Uh, "bh": bh,
        })
    res = run_bass_kernel_spmd(nc, in_maps, list(range(N_CORES)))
    h_out = np.concatenate([res.results[c]["h_out"] for c in range(N_CORES)],
                           axis=0)
    k_out = np.concatenate([res.results[c]["k_out"] for c in range(N_CORES)],
                           axis=0)
    kp_out = np.concatenate([res.results[c]["kp_out"]
                             for c in range(N_CORES)], axis=0)
    return (h_out, k_out, kp_out)
